# revision 11
# baseline (speedup 1.0000x reference)
"""Trainium2 Bass kernel for the BH4 butterfly module.

The reference computes, per token x (row vector, D=1024):
    y = DECAY * bh4(x, w) + (1-DECAY) * tile(x, R), truncated to 4096, + bias
where bh4 applies, for each repeat r, 4 rounds of (block-diagonal matmul with
16 blocks of 64x64, then a (16,64)-grid transpose permutation of the features).

Each repeat's 4-layer butterfly chain composes into a single dense 1024x1024
matrix A_r (the product of butterfly factors is dense), so the whole module is
one GEMM:
    y = x @ W + 0.3*tile(x, R) + bias,   W = 0.7*[A_0 | A_1 | A_2 | A_3]
W is composed on the host in float64 from the `weight` input (cheap: ~2 GFLOP),
and the GEMM runs on the TensorEngine in fp8-e4m3 with DoubleRow perf mode,
accumulating in fp32 PSUM. A dynamic power-of-2 rescale keeps the tiny composed
weights above e4m3's subnormal floor and is undone exactly on the host.

The kernel is DMA-bandwidth-bound (~360 GB/s serialized across all queues), so
the two big-ticket tensors ride in 16-bit: the skip term (1-DECAY)*x is loaded
as bf16 and the output is stored as bf16 and upcast on the host. The butterfly
term is ~1e-6 of the output (the reference's weight normalization shrinks
variance ~1024x per layer), so output precision is set by the bf16 skip path:
measured rel err ~1.7e-3 against the fp32 reference, far inside tolerance.
Per-core traffic: xt 1MB fp8 + W 4MB fp8 + resid 2MB bf16 in, y 8MB bf16 out
= 15.7MB -> ~43.7us of transfer, which the schedule keeps gapless: single
SP DMA queue ordered xt, W0-1, resid singles (streaming just ahead of the
evacuation cadence), W2-7, then the 512KB paired output stores; dummy PE
matmuls bridge the p-state ramp before the first operands land and across
the one W-block bubble; PSUM evacuation alternates fused DVE adds with
ACT-copy + DVE 4x-bf16 adds (one per npair on GpSimd) so banks recycle
faster than the PE's 853ns/group. Cost-model makespan 47.3us vs the 47.2us
head + transfer + tail floor (baseline: 76.6us).

Sharding: data-parallel over the 8192 flattened tokens -> 1024 tokens/core on
8 NeuronCores; W and bias replicated.
"""

import numpy as np
import ml_dtypes

D = 1024          # in_dim
R = 4             # num_repeat
OUT_DIM = 4096
DECAY = 0.7
N_CORES = 8
P = 128           # partitions

_BASS_CACHE = {}
LAST_EXEC_TIME_NS = None


def _compose_dense(weight: np.ndarray) -> np.ndarray:
    """weight [R, 4, NB, BS, BS] -> dense [D, R*D] with bh4(x, w) == x @ A."""
    R_, L, NB, BS, _ = weight.shape
    d = NB * BS
    w = weight.astype(np.float64)
    mats = []
    for r in range(R_):
        E = np.eye(d, dtype=np.float64)
        for k in range(L):
            Eb = E.reshape(d, NB, BS).transpose(1, 0, 2)   # [NB, d, BS]
            Eb = np.matmul(Eb, w[r, k])                    # [NB, d, BS]
            E = Eb.transpose(1, 0, 2)                      # [d, NB, BS]
            E = E.transpose(0, 2, 1).reshape(d, d)         # col n*BS+i -> i*NB+n
        mats.append(E)
    return np.concatenate(mats, axis=1)


def _build_bass(tokens_per_core: int, with_bias: bool = True):
    """Build the SPMD Bass program for one core's GEMM + skip (+ bias)."""
    import concourse.bacc as bacc
    import concourse.mybir as mybir
    import concourse.tile as tile
    from concourse.bass import ts

    T = tokens_per_core
    KT = D // P                 # 8 k-tiles of 128
    MT = T // P                 # 8 token tiles of 128
    NP = OUT_DIM // 1024        # 4 output column pairs of 1024
    mm_dt = mybir.dt.float8e4

    nc = bacc.Bacc("TRN2", target_bir_lowering=False, debug=False, num_devices=N_CORES)
    xt = nc.dram_tensor("xt", [D, T], mm_dt, kind="ExternalInput")
    w = nc.dram_tensor("w", [D, OUT_DIM], mm_dt, kind="ExternalInput")
    resid = nc.dram_tensor("resid", [T, D], mybir.dt.bfloat16, kind="ExternalInput")
    bias = nc.dram_tensor("bias", [OUT_DIM], mybir.dt.float32, kind="ExternalInput")
    y = nc.dram_tensor("y", [T, OUT_DIM], mybir.dt.bfloat16, kind="ExternalOutput")

    xt_r = xt.ap().rearrange("(ko p) t -> p ko t", p=P)
    w_r = w.ap().rearrange("(ko p) n -> p ko n", p=P)
    resid_r = resid.ap().rearrange("(mt p) c -> p mt c", p=P)
    y_r = y.ap().rearrange("(mt p) n -> p mt n", p=P)

    with tile.TileContext(nc) as tc:
        with (
            tc.tile_pool(name="const", bufs=1) as const_pool,
            tc.tile_pool(name="psum", bufs=4, space="PSUM") as psum_pool,
            tc.tile_pool(name="out", bufs=16) as out_pool,
        ):
            # All loads AND stores ride the single SP HWDGE queue: the DMA
            # engines are one serialized bandwidth pool, so a second queue
            # only scrambles the order. Load order: the PE's critical path
            # (xt then the W column blocks, consumed npair by npair) streams
            # first; resid singles slot in late between the last W blocks -
            # the in-place resid adds are decoupled from PSUM evacuation, so
            # resid is only needed just before each tile's store slot. All
            # transfers keep >=512B contiguous runs per partition.
            xt_sb = const_pool.tile([P, KT, T], mm_dt)
            w_sb = const_pool.tile([P, KT, OUT_DIM], mm_dt)
            resid_sb = const_pool.tile([P, MT, D], mybir.dt.bfloat16)

            def load_w(n):
                nc.sync.dma_start(w_sb[:, :, ts(n, 512)], w_r[:, :, ts(n, 512)])

            def load_r(m):
                nc.sync.dma_start(resid_sb[:, m], resid_r[:, m])

            # xt + W blocks 0/1 first (PE critical path, GEMM starts ~8.7us),
            # then the resid singles: at 728ns each they stream just ahead of
            # np0's evacuation cadence (853ns/group), so the fused adds never
            # wait. W blocks 2-7 follow; np1 starts ~1.9us late (bridged by
            # dummy matmuls below), np2/np3 arrivals beat the PE comfortably.
            nc.sync.dma_start(xt_sb[:], xt_r[:])
            load_w(0)
            load_w(1)
            for m in range(MT):
                load_r(m)
            for n in range(2, 8):
                load_w(n)

            if with_bias:
                bias_stage = const_pool.tile([1, OUT_DIM], mybir.dt.float32)
                bias_sb = const_pool.tile([P, OUT_DIM], mybir.dt.float32)
                nc.scalar.dma_start(bias_stage[:], bias.ap()[None, :])
                nc.gpsimd.partition_broadcast(bias_sb[:], bias_stage[:])

            # PE p-state warmup: the cost model (and HW) ramps the Tensor
            # engine 0.65 -> 1.2 -> 2.4 GHz over ~3us of *continuous* work;
            # any idle gap resets the ramp. Dummy DoubleRow matmuls on a
            # zeroed tile keep the PE busy from t~0.5us until the first real
            # operands (xt + W blocks 0/1) land at ~8.7us, so the whole GEMM
            # runs at full clock. Results go to a psum bank never read.
            warm = const_pool.tile([P, 2, 512], mm_dt)
            nc.gpsimd.memset(warm[:], 0)
            ps_w = psum_pool.tile([P, 1024], mybir.dt.float32, tag="grp")
            for _ in range(50):
                nc.tensor.matmul(
                    ps_w[:, ts(0, 512)],
                    warm[:, :, :P],
                    warm[:, :, :],
                    start=True,
                    stop=True,
                    perf_mode=mybir.MatmulPerfMode.DoubleRow,
                )

            # Tile groups: (npair, m) -> psum [P, 1024] spanning two banks
            # (each half written by 4 DoubleRow matmuls contracting K=256),
            # npair-major so the first groups touch only W blocks 0-1 and the
            # PE can start while the rest of W streams in.
            #
            # Evacuation (fused, resid arrives just in time): odd m -> one
            # fused psum+resid add on DVE; even m -> ACT copy (psum -> bf16)
            # then an all-SBUF bf16 in-place add (DVE 4x mode; the m==6 add
            # rides the otherwise-idle GpSimd). Per npair that is ~6.5us of
            # DVE work against the PE's 6.8us - every PSUM bank returns
            # within ~1.2us and the PE never waits. Stores are 512KB m-pairs
            # on SP behind the loads; the last pair goes as two singles.
            ot_tiles = {}
            for npair in range(NP):
                if npair == 1:
                    # W block 2/3 land ~1.9us after np0's matmuls finish;
                    # keep the p-state ramp alive across that bubble. Fresh
                    # pool tile: ps_w's bank belongs to group 7 by now, and
                    # the pool only serializes accesses for tiles it rotates.
                    ps_w2 = psum_pool.tile([P, 1024], mybir.dt.float32, tag="grp")
                    for _ in range(18):
                        nc.tensor.matmul(
                            ps_w2[:, ts(0, 512)],
                            warm[:, :, :P],
                            warm[:, :, :],
                            start=True,
                            stop=True,
                            perf_mode=mybir.MatmulPerfMode.DoubleRow,
                        )
                for m in range(MT):
                    ps = psum_pool.tile([P, 1024], mybir.dt.float32, tag="grp")
                    for half in range(2):
                        n = 2 * npair + half
                        for kk in range(0, KT, 2):
                            nc.tensor.matmul(
                                ps[:, ts(half, 512)],
                                xt_sb[:, kk : kk + 2, ts(m, P)],
                                w_sb[:, kk : kk + 2, ts(n, 512)],
                                start=(kk == 0),
                                stop=(kk == KT - 2),
                                perf_mode=mybir.MatmulPerfMode.DoubleRow,
                            )
                    mp = m // 2
                    if m % 2 == 0:
                        ot_tiles[(npair, mp)] = out_pool.tile(
                            [P, 2, 1024], mybir.dt.bfloat16, name="ot", tag="ot"
                        )
                    ot = ot_tiles[(npair, mp)]
                    if m % 2 == 0:
                        nc.scalar.copy(ot[:, 0], ps[:])
                        eng = nc.gpsimd if m == 6 else nc.vector
                        eng.tensor_add(ot[:, 0], ot[:, 0], resid_sb[:, m])
                    else:
                        nc.vector.tensor_add(
                            ot[:, 1], ps[:], resid_sb[:, m]
                        )
                    if with_bias:
                        nc.gpsimd.tensor_add(
                            ot[:, m % 2], ot[:, m % 2], bias_sb[:, ts(npair, 1024)]
                        )
                    if m % 2 == 1:
                        last_pair = npair == NP - 1 and mp == MT // 2 - 1
                        if last_pair:
                            nc.sync.dma_start(
                                y_r[:, 2 * mp, ts(npair, 1024)], ot[:, 0]
                            )
                            nc.sync.dma_start(
                                y_r[:, 2 * mp + 1, ts(npair, 1024)], ot[:, 1]
                            )
                        else:
                            nc.sync.dma_start(
                                y_r[:, 2 * mp : 2 * mp + 2, ts(npair, 1024)],
                                ot[:],
                            )

    nc.compile()
    return nc


def _run(inputs: dict, trace: bool = False):
    from concourse.bass_utils import run_bass_kernel_spmd

    xs = np.asarray(inputs["xs"])
    weight = np.asarray(inputs["weight"])
    bias = np.asarray(inputs["bias"], dtype=np.float32)

    lead_shape = xs.shape[:-1]
    xf = np.ascontiguousarray(xs.reshape(-1, D), dtype=np.float32)
    n_tok = xf.shape[0]
    assert n_tok % N_CORES == 0
    tpc = n_tok // N_CORES

    # host compose: dense butterfly matrix, scaled by DECAY
    w_dense = DECAY * _compose_dense(weight)[:, :OUT_DIM]
    # Power-of-2 rescale for fp8: the composed butterfly weights here are
    # ~2e-8 (the reference's normalization shrinks them ~1024x per layer),
    # far below e4m3's subnormal floor. Bring amax to ~2^7 on device and
    # undo it exactly (fp32 exponent shift) on the host after gathering.
    amax = float(np.abs(w_dense).max())
    exp = int(np.clip(np.floor(np.log2(128.0 / amax)), -120, 120)) if amax > 0 else 0
    scale = float(2.0 ** exp)
    w_dev = (w_dense * scale).astype(np.float32).astype(ml_dtypes.float8_e4m3)
    bias_dev = np.ascontiguousarray(bias * scale, dtype=np.float32)

    with_bias = bool(np.any(bias != 0.0))
    key = (tpc, with_bias)
    if key not in _BASS_CACHE:
        _BASS_CACHE[key] = _build_bass(tpc, with_bias=with_bias)
    nc = _BASS_CACHE[key]

    in_maps = []
    for c in range(N_CORES):
        xc = xf[c * tpc : (c + 1) * tpc]                    # [tpc, D] fp32
        in_maps.append(
            {
                "xt": np.ascontiguousarray(xc.T).astype(ml_dtypes.float8_e4m3),
                "w": w_dev,
                "resid": ((1.0 - DECAY) * scale * xc).astype(ml_dtypes.bfloat16),
                "bias": bias_dev,
            }
        )

    # The axon-tunneled terminal intermittently reports
    # NRT_EXEC_UNIT_UNRECOVERABLE; the immediately-following run always
    # succeeded. Retry with a backend reset.
    last_exc = None
    for attempt in range(3):
        try:
            res = run_bass_kernel_spmd(
                nc, in_maps, core_ids=list(range(N_CORES)), trace=trace
            )
            break
        except Exception as e:  # noqa: BLE001 - device fault -> reset + retry
            last_exc = e
            try:
                import jax
                import jax.extend

                jax.clear_caches()
                jax.extend.backend.clear_backends()
            except Exception:
                pass
    else:
        raise last_exc
    global LAST_EXEC_TIME_NS
    LAST_EXEC_TIME_NS = res.exec_time_ns

    y = np.concatenate(
        [np.asarray(r["y"]).astype(np.float32) for r in res.results], axis=0
    )
    if scale != 1.0:
        y = y * np.float32(1.0 / scale)   # exact: power-of-2 exponent shift
    return y.reshape(*lead_shape, OUT_DIM), res


def kernel(**inputs) -> np.ndarray:
    out, _ = _run(inputs, trace=False)
    return out


# revision 12
# speedup vs baseline: 1.0324x; 1.0324x over previous
"""Trainium2 Bass kernel for the BH4 butterfly module.

The reference computes, per token x (row vector, D=1024):
    y = DECAY * bh4(x, w) + (1-DECAY) * tile(x, R), truncated to 4096, + bias
where bh4 applies, for each repeat r, 4 rounds of (block-diagonal matmul with
16 blocks of 64x64, then a (16,64)-grid transpose permutation of the features).

Each repeat's 4-layer butterfly chain composes into a single dense 1024x1024
matrix A_r (the product of butterfly factors is dense), so the whole module is
one GEMM:
    y = x @ W + 0.3*tile(x, R) + bias,   W = 0.7*[A_0 | A_1 | A_2 | A_3]
W is composed on the host in float64 from the `weight` input (cheap: ~2 GFLOP),
and the GEMM runs on the TensorEngine in fp8-e4m3 with DoubleRow perf mode,
accumulating in fp32 PSUM. A dynamic power-of-2 rescale keeps the tiny composed
weights above e4m3's subnormal floor and is undone exactly on the host.

The kernel is DMA-bandwidth-bound (~360 GB/s serialized across all queues), so
the kernel computes y TRANSPOSED (output features on partitions): the skip
term (1-DECAY)*x^T then has the same [D, T] layout as the GEMM's moving
operand, so x ships ONCE as bf16 (2MB) and the fp8 moving operand is derived
on-device by a cheap SBUF->SBUF scaled cast spread over ACT/DVE/GpSimd before
their evacuation duties begin. The output stores as bf16 and is transposed +
upcast on the host. The butterfly term is ~1e-6 of the output (the reference's
weight normalization shrinks variance ~1024x per layer), so output precision
is set by the bf16 skip path: rel err ~1.7e-3, far inside tolerance.

Per-core traffic: x^T 2MB bf16 + W 4MB fp8 in, y^T 8MB bf16 out = 14.7MB ->
~40.8us of transfer, kept gapless: one SP DMA queue ordered x^T slices (so
casts start immediately), W column blocks (arriving ahead of the PE's
npair-major consumption), then 512KB paired output stores. Dummy PE matmuls
hold the p-state ramp at 2.4GHz until the casts land (~10us); PSUM evacuation
alternates fused DVE adds with ACT-copy + DVE 4x-bf16 adds (one per octet on
GpSimd) so banks recycle faster than the PE's 853ns/group. Cost-model
makespan ~44.4us vs the 47.3us of the untransposed layout (x shipped twice)
and 76.6us baseline.

Sharding: data-parallel over the 8192 flattened tokens -> 1024 tokens/core on
8 NeuronCores; W and bias replicated.
"""

import numpy as np
import ml_dtypes

D = 1024          # in_dim
R = 4             # num_repeat
OUT_DIM = 4096
DECAY = 0.7
N_CORES = 8
P = 128           # partitions

_BASS_CACHE = {}
LAST_EXEC_TIME_NS = None


def _compose_dense(weight: np.ndarray) -> np.ndarray:
    """weight [R, 4, NB, BS, BS] -> dense [D, R*D] with bh4(x, w) == x @ A."""
    R_, L, NB, BS, _ = weight.shape
    d = NB * BS
    w = weight.astype(np.float64)
    mats = []
    for r in range(R_):
        E = np.eye(d, dtype=np.float64)
        for k in range(L):
            Eb = E.reshape(d, NB, BS).transpose(1, 0, 2)   # [NB, d, BS]
            Eb = np.matmul(Eb, w[r, k])                    # [NB, d, BS]
            E = Eb.transpose(1, 0, 2)                      # [d, NB, BS]
            E = E.transpose(0, 2, 1).reshape(d, d)         # col n*BS+i -> i*NB+n
        mats.append(E)
    return np.concatenate(mats, axis=1)


def _build_bass(tokens_per_core: int, exp: int, with_bias: bool = True):
    """Build the SPMD Bass program for one core's transposed GEMM + skip."""
    import concourse.bacc as bacc
    import concourse.mybir as mybir
    import concourse.tile as tile
    from concourse.bass import ts

    T = tokens_per_core
    KT = D // P                 # 8 k-tiles of 128 (input features)
    OB = OUT_DIM // P           # 32 output-feature blocks of 128
    mm_dt = mybir.dt.float8e4
    # xb holds (1-DECAY)*2^exp*x^T in bf16; the cast to the fp8 GEMM operand
    # rescales to ~unit variance (exactness is irrelevant on the fp8 path)
    cast_imm = float(2.0 ** (-exp) / (1.0 - DECAY))

    nc = bacc.Bacc("TRN2", target_bir_lowering=False, debug=False, num_devices=N_CORES)
    xb = nc.dram_tensor("xb", [D, T], mybir.dt.bfloat16, kind="ExternalInput")
    w = nc.dram_tensor("w", [D, OUT_DIM], mm_dt, kind="ExternalInput")
    bias = nc.dram_tensor("bias", [OUT_DIM], mybir.dt.float32, kind="ExternalInput")
    yt = nc.dram_tensor("yt", [OUT_DIM, T], mybir.dt.bfloat16, kind="ExternalOutput")

    xb_r = xb.ap().rearrange("(ko p) t -> p ko t", p=P)
    w_r = w.ap().rearrange("(ko p) n -> p ko n", p=P)
    yt_r = yt.ap().rearrange("(ob p) t -> p ob t", p=P)

    with tile.TileContext(nc) as tc:
        with (
            tc.tile_pool(name="const", bufs=1) as const_pool,
            tc.tile_pool(name="psum", bufs=4, space="PSUM") as psum_pool,
            tc.tile_pool(name="out", bufs=16) as out_pool,
        ):
            # All loads AND stores ride the single SP HWDGE queue (the DMA
            # engines are one serialized bandwidth pool; a second queue only
            # scrambles the order). x^T slices go first so the fp8 casts can
            # start immediately; W blocks follow, each arriving well before
            # the PE's npair-major consumption reaches it. All transfers keep
            # >=512B contiguous runs per partition.
            xb_sb = const_pool.tile([P, KT, T], mybir.dt.bfloat16)
            w_sb = const_pool.tile([P, KT, OUT_DIM], mm_dt)
            xf8 = const_pool.tile([P, KT, T], mm_dt)

            for k in range(KT):
                nc.sync.dma_start(xb_sb[:, k], xb_r[:, k])
            for n in range(OUT_DIM // 512):
                nc.sync.dma_start(w_sb[:, :, ts(n, 512)], w_r[:, :, ts(n, 512)])

            # fp8 cast of the moving operand, spread over the three non-PE
            # engines (all finish by ~10us, before evacuation work begins;
            # GpSimd gets the two middle slices - it is idle and slow)
            for k in range(KT):
                sel = {0: "a", 3: "a", 6: "a", 1: "v", 4: "v", 7: "v"}.get(k, "p")
                if sel == "a":
                    nc.scalar.mul(xf8[:, k], xb_sb[:, k], cast_imm)
                elif sel == "v":
                    nc.vector.tensor_scalar_mul(xf8[:, k], xb_sb[:, k], cast_imm)
                else:
                    nc.gpsimd.tensor_scalar_mul(xf8[:, k], xb_sb[:, k], cast_imm)

            if with_bias:
                # bias along the PARTITION dim now: [P, OB] fp32, one
                # per-partition scalar column per oc block
                bias_sb = const_pool.tile([P, OB], mybir.dt.float32)
                nc.scalar.dma_start(
                    bias_sb[:], bias.ap().rearrange("(ob p) -> p ob", p=P)
                )

            # PE p-state warmup: the cost model (and HW) ramps the Tensor
            # engine 0.65 -> 1.2 -> 2.4 GHz over ~3us of *continuous* work;
            # any idle gap resets the ramp. Dummy DoubleRow matmuls on a
            # zeroed tile keep the PE busy from t~1.7us until the casts land
            # (~10.2us), so the whole GEMM runs at full clock.
            warm = const_pool.tile([P, 2, 512], mm_dt)
            nc.gpsimd.memset(warm[:], 0)
            ps_w = psum_pool.tile([P, 1024], mybir.dt.float32, tag="grp")
            for _ in range(66):
                nc.tensor.matmul(
                    ps_w[:, ts(0, 512)],
                    warm[:, :, :P],
                    warm[:, :, :],
                    start=True,
                    stop=True,
                    perf_mode=mybir.MatmulPerfMode.DoubleRow,
                )

            # Tile groups: one per 128-wide output-feature block g ->
            # psum [P(oc), 1024 tokens] spanning two banks, filled by
            # 2 token-blocks x 4 DoubleRow matmuls (stationary = W column
            # slice, moving = xf8). The skip operand for block g is simply
            # xb_sb[:, g % KT, :] - same layout, no transpose needed.
            #
            # Evacuation (all skip operands resident from ~10us): odd g ->
            # fused psum+skip add on DVE; even g -> ACT copy then an all-SBUF
            # bf16 in-place add (DVE 4x mode; the g%8==6 add rides GpSimd).
            # Banks recycle ~1.2us after each group vs the PE's 853ns/group
            # cadence with 4 groups in flight - the PE never waits. Stores
            # are 512KB g-pairs on SP behind the loads; last pair as singles.
            ot_tiles = {}
            for g in range(OB):
                ps = psum_pool.tile([P, 1024], mybir.dt.float32, tag="grp")
                for tb in range(2):
                    for kk in range(0, KT, 2):
                        nc.tensor.matmul(
                            ps[:, ts(tb, 512)],
                            w_sb[:, kk : kk + 2, ts(g, P)],
                            xf8[:, kk : kk + 2, ts(tb, 512)],
                            start=(kk == 0),
                            stop=(kk == KT - 2),
                            perf_mode=mybir.MatmulPerfMode.DoubleRow,
                        )
                gp = g // 2
                if g % 2 == 0:
                    ot_tiles[gp] = out_pool.tile(
                        [P, 2, 1024], mybir.dt.bfloat16, name="ot", tag="ot"
                    )
                ot = ot_tiles[gp]
                skip = xb_sb[:, g % KT, :]
                if g % 2 == 0:
                    nc.scalar.copy(ot[:, 0], ps[:])
                    eng = nc.gpsimd if g % 8 == 6 else nc.vector
                    eng.tensor_add(ot[:, 0], ot[:, 0], skip)
                else:
                    nc.vector.tensor_add(ot[:, 1], ps[:], skip)
                if with_bias:
                    nc.gpsimd.tensor_scalar_add(
                        ot[:, g % 2], ot[:, g % 2], bias_sb[:, g : g + 1]
                    )
                if g % 2 == 1:
                    if g == OB - 1:
                        nc.sync.dma_start(yt_r[:, g - 1], ot[:, 0])
                        nc.sync.dma_start(yt_r[:, g], ot[:, 1])
                    else:
                        nc.sync.dma_start(yt_r[:, g - 1 : g + 1], ot[:])

    nc.compile()
    return nc


def _run(inputs: dict, trace: bool = False):
    from concourse.bass_utils import run_bass_kernel_spmd

    xs = np.asarray(inputs["xs"])
    weight = np.asarray(inputs["weight"])
    bias = np.asarray(inputs["bias"], dtype=np.float32)

    lead_shape = xs.shape[:-1]
    xf = np.ascontiguousarray(xs.reshape(-1, D), dtype=np.float32)
    n_tok = xf.shape[0]
    assert n_tok % N_CORES == 0
    tpc = n_tok // N_CORES

    # host compose: dense butterfly matrix, scaled by DECAY
    w_dense = DECAY * _compose_dense(weight)[:, :OUT_DIM]
    # Power-of-2 rescale for fp8: the composed butterfly weights here are
    # ~2e-8 (the reference's normalization shrinks them ~1024x per layer),
    # far below e4m3's subnormal floor. Bring amax to ~2^7 on device and
    # undo it exactly (fp32 exponent shift) on the host after gathering.
    amax = float(np.abs(w_dense).max())
    exp = int(np.clip(np.floor(np.log2(128.0 / amax)), -120, 120)) if amax > 0 else 0
    scale = float(2.0 ** exp)
    w_dev = (w_dense * scale).astype(np.float32).astype(ml_dtypes.float8_e4m3)
    bias_dev = np.ascontiguousarray(bias * scale, dtype=np.float32)

    with_bias = bool(np.any(bias != 0.0))
    key = (tpc, exp, with_bias)
    if key not in _BASS_CACHE:
        _BASS_CACHE[key] = _build_bass(tpc, exp, with_bias=with_bias)
    nc = _BASS_CACHE[key]

    in_maps = []
    for c in range(N_CORES):
        xc = xf[c * tpc : (c + 1) * tpc]                    # [tpc, D] fp32
        in_maps.append(
            {
                "xb": np.ascontiguousarray(
                    ((1.0 - DECAY) * scale * xc).T
                ).astype(ml_dtypes.bfloat16),
                "w": w_dev,
                "bias": bias_dev,
            }
        )

    # The axon-tunneled terminal intermittently reports
    # NRT_EXEC_UNIT_UNRECOVERABLE; the immediately-following run always
    # succeeded. Retry with a backend reset.
    last_exc = None
    for attempt in range(3):
        try:
            res = run_bass_kernel_spmd(
                nc, in_maps, core_ids=list(range(N_CORES)), trace=trace
            )
            break
        except Exception as e:  # noqa: BLE001 - device fault -> reset + retry
            last_exc = e
            try:
                import jax
                import jax.extend

                jax.clear_caches()
                jax.extend.backend.clear_backends()
            except Exception:
                pass
    else:
        raise last_exc
    global LAST_EXEC_TIME_NS
    LAST_EXEC_TIME_NS = res.exec_time_ns

    y = np.concatenate(
        [np.asarray(r["yt"]).T.astype(np.float32) for r in res.results], axis=0
    )
    if scale != 1.0:
        y = y * np.float32(1.0 / scale)   # exact: power-of-2 exponent shift
    return y.reshape(*lead_shape, OUT_DIM), res


def kernel(**inputs) -> np.ndarray:
    out, _ = _run(inputs, trace=False)
    return out


# revision 13
# speedup vs baseline: 1.0662x; 1.0327x over previous
"""Trainium2 Bass kernel for the BH4 butterfly module.

The reference computes, per token x (row vector, D=1024):
    y = DECAY * bh4(x, w) + (1-DECAY) * tile(x, R), truncated to 4096, + bias
where bh4 applies, for each repeat r, 4 rounds of (block-diagonal matmul with
16 blocks of 64x64, then a (16,64)-grid transpose permutation of the features).

Each repeat's 4-layer butterfly chain composes into a single dense 1024x1024
matrix A_r (the product of butterfly factors is dense), so the whole module is
one GEMM:
    y = x @ W + 0.3*tile(x, R) + bias,   W = 0.7*[A_0 | A_1 | A_2 | A_3]
W is composed on the host in float64 from the `weight` input (cheap: ~2 GFLOP),
and the GEMM runs on the TensorEngine in fp8-e4m3 with DoubleRow perf mode,
accumulating in fp32 PSUM. A dynamic power-of-2 rescale keeps the tiny composed
weights above e4m3's subnormal floor and is undone exactly on the host.

The kernel is DMA-bandwidth-bound (~360 GB/s serialized across all queues), so
the kernel computes y TRANSPOSED (output features on partitions): the skip
term (1-DECAY)*x^T then has the same [D, T] layout as the GEMM's moving
operand, so x ships ONCE as bf16 (2MB) and the fp8 moving operand is derived
on-device by a cheap SBUF->SBUF scaled cast spread over ACT/DVE/GpSimd before
their evacuation duties begin. The output stores as bf16 and is transposed +
upcast on the host. The butterfly term is ~1e-6 of the output (the reference's
weight normalization shrinks variance ~1024x per layer), so output precision
is set by the bf16 skip path: rel err ~1.7e-3, far inside tolerance.

Per-core traffic: x^T 2MB bf16 + W 4MB fp8 in, y^T 8MB bf16 out = 14.7MB ->
~40.8us of transfer, kept gapless: one SP DMA queue ordered x^T slices (so
casts start immediately), W column blocks (arriving ahead of the PE's
npair-major consumption), then 512KB paired output stores. Dummy PE matmuls
hold the p-state ramp at 2.4GHz until the casts land (~10us); PSUM evacuation
alternates fused DVE adds with ACT-copy + DVE 4x-bf16 adds (one per octet on
GpSimd) so banks recycle faster than the PE's 853ns/group. Cost-model
makespan ~44.4us vs the 47.3us of the untransposed layout (x shipped twice)
and 76.6us baseline.

Sharding: data-parallel over the 8192 flattened tokens -> 1024 tokens/core on
8 NeuronCores; W and bias replicated.
"""

import numpy as np
import ml_dtypes

D = 1024          # in_dim
R = 4             # num_repeat
OUT_DIM = 4096
DECAY = 0.7
N_CORES = 8
P = 128           # partitions

_BASS_CACHE = {}
LAST_EXEC_TIME_NS = None


def _compose_dense(weight: np.ndarray) -> np.ndarray:
    """weight [R, 4, NB, BS, BS] -> dense [D, R*D] with bh4(x, w) == x @ A."""
    R_, L, NB, BS, _ = weight.shape
    d = NB * BS
    w = weight.astype(np.float64)
    mats = []
    for r in range(R_):
        E = np.eye(d, dtype=np.float64)
        for k in range(L):
            Eb = E.reshape(d, NB, BS).transpose(1, 0, 2)   # [NB, d, BS]
            Eb = np.matmul(Eb, w[r, k])                    # [NB, d, BS]
            E = Eb.transpose(1, 0, 2)                      # [d, NB, BS]
            E = E.transpose(0, 2, 1).reshape(d, d)         # col n*BS+i -> i*NB+n
        mats.append(E)
    return np.concatenate(mats, axis=1)


def _build_bass(tokens_per_core: int, exp: int, with_bias: bool = True):
    """Build the SPMD Bass program for one core's transposed GEMM + skip."""
    import concourse.bacc as bacc
    import concourse.mybir as mybir
    import concourse.tile as tile
    from concourse.bass import ts

    T = tokens_per_core
    KT = D // P                 # 8 k-tiles of 128 (input features)
    OB = OUT_DIM // P           # 32 output-feature blocks of 128
    mm_dt = mybir.dt.float8e4
    # xb holds (1-DECAY)*2^exp*x^T in bf16; the cast to the fp8 GEMM operand
    # rescales to ~unit variance (exactness is irrelevant on the fp8 path)
    cast_imm = float(2.0 ** (-exp) / (1.0 - DECAY))

    nc = bacc.Bacc("TRN2", target_bir_lowering=False, debug=False, num_devices=N_CORES)
    xb = nc.dram_tensor("xb", [D, T], mybir.dt.bfloat16, kind="ExternalInput")
    w = nc.dram_tensor("w", [D, OUT_DIM], mm_dt, kind="ExternalInput")
    bias = nc.dram_tensor("bias", [OUT_DIM], mybir.dt.float32, kind="ExternalInput")
    yt = nc.dram_tensor("yt", [OUT_DIM, T], mybir.dt.bfloat16, kind="ExternalOutput")

    xb_r = xb.ap().rearrange("(ko p) t -> p ko t", p=P)
    w_r = w.ap().rearrange("(ko p) n -> p ko n", p=P)
    yt_r = yt.ap().rearrange("(ob p) t -> p ob t", p=P)

    with tile.TileContext(nc) as tc:
        with (
            tc.tile_pool(name="const", bufs=1) as const_pool,
            tc.tile_pool(name="psum", bufs=4, space="PSUM") as psum_pool,
            tc.tile_pool(name="out", bufs=16) as out_pool,
        ):
            # All loads AND stores ride the single SP HWDGE queue (the DMA
            # engines are one serialized bandwidth pool; a second queue only
            # scrambles the order). x^T slices go first so the fp8 casts can
            # start immediately; W blocks follow, each arriving well before
            # the PE's npair-major consumption reaches it. All transfers keep
            # >=512B contiguous runs per partition.
            xb_sb = const_pool.tile([P, KT, T], mybir.dt.bfloat16)
            w_sb = const_pool.tile([P, KT, OUT_DIM], mm_dt)
            xf8 = const_pool.tile([P, KT, T], mm_dt)

            nc.sync.dma_start(w_sb[:, :, ts(0, 512)], w_r[:, :, ts(0, 512)])
            for k in range(KT):
                nc.sync.dma_start(xb_sb[:, k], xb_r[:, k])
            for n in range(1, OUT_DIM // 512):
                nc.sync.dma_start(w_sb[:, :, ts(n, 512)], w_r[:, :, ts(n, 512)])

            # fp8 cast of the moving operand, spread over the three non-PE
            # engines (all finish by ~10us, before evacuation work begins;
            # GpSimd gets the two middle slices - it is idle and slow)
            for k in range(KT):
                sel = {0: "a", 3: "a", 6: "a", 1: "v", 4: "v", 7: "v"}.get(k, "p")
                if sel == "a":
                    nc.scalar.mul(xf8[:, k], xb_sb[:, k], cast_imm)
                elif sel == "v":
                    nc.vector.tensor_scalar_mul(xf8[:, k], xb_sb[:, k], cast_imm)
                else:
                    nc.gpsimd.tensor_scalar_mul(xf8[:, k], xb_sb[:, k], cast_imm)

            if with_bias:
                # bias along the PARTITION dim now: [P, OB] fp32, one
                # per-partition scalar column per oc block
                bias_sb = const_pool.tile([P, OB], mybir.dt.float32)
                nc.scalar.dma_start(
                    bias_sb[:], bias.ap().rearrange("(ob p) -> p ob", p=P)
                )

            # PE p-state warmup: the cost model (and HW) ramps the Tensor
            # engine 0.65 -> 1.2 -> 2.4 GHz over ~3us of *continuous* work;
            # any idle gap resets the ramp. Dummy DoubleRow matmuls on a
            # zeroed tile keep the PE busy from t~1.7us until the casts land
            # (~10.2us), so the whole GEMM runs at full clock.
            warm = const_pool.tile([P, 2, 512], mm_dt)
            nc.gpsimd.memset(warm[:], 0)
            ps_w = psum_pool.tile([P, 1024], mybir.dt.float32, tag="grp")
            for _ in range(57):
                nc.tensor.matmul(
                    ps_w[:, ts(0, 512)],
                    warm[:, :, :P],
                    warm[:, :, :],
                    start=True,
                    stop=True,
                    perf_mode=mybir.MatmulPerfMode.DoubleRow,
                )

            # Tile groups: one per 128-wide output-feature block g ->
            # psum [P(oc), 1024 tokens] spanning two banks, filled by
            # 2 token-blocks x 4 DoubleRow matmuls (stationary = W column
            # slice, moving = xf8). The skip operand for block g is simply
            # xb_sb[:, g % KT, :] - same layout, no transpose needed.
            #
            # Evacuation (all skip operands resident from ~10us): odd g ->
            # fused psum+skip add on DVE; even g -> ACT copy then an all-SBUF
            # bf16 in-place add (DVE 4x mode; the g%8==6 add rides GpSimd).
            # Banks recycle ~1.2us after each group vs the PE's 853ns/group
            # cadence with 4 groups in flight - the PE never waits. Stores
            # are 512KB g-pairs on SP behind the loads; last pair as singles.
            ot_tiles = {}
            for g in range(OB):
                ps = psum_pool.tile([P, 1024], mybir.dt.float32, tag="grp")
                for tb in range(2):
                    for kk in range(0, KT, 2):
                        nc.tensor.matmul(
                            ps[:, ts(tb, 512)],
                            w_sb[:, kk : kk + 2, ts(g, P)],
                            xf8[:, kk : kk + 2, ts(tb, 512)],
                            start=(kk == 0),
                            stop=(kk == KT - 2),
                            perf_mode=mybir.MatmulPerfMode.DoubleRow,
                        )
                gp = g // 2
                if g % 2 == 0:
                    ot_tiles[gp] = out_pool.tile(
                        [P, 2, 1024], mybir.dt.bfloat16, name="ot", tag="ot"
                    )
                ot = ot_tiles[gp]
                skip = xb_sb[:, g % KT, :]
                if g % 2 == 0:
                    nc.scalar.copy(ot[:, 0], ps[:])
                    eng = nc.gpsimd if (g % 8 == 6 and g < 24) else nc.vector
                    eng.tensor_add(ot[:, 0], ot[:, 0], skip)
                else:
                    nc.vector.tensor_add(ot[:, 1], ps[:], skip)
                if with_bias:
                    nc.gpsimd.tensor_scalar_add(
                        ot[:, g % 2], ot[:, g % 2], bias_sb[:, g : g + 1]
                    )
                if g % 2 == 1:
                    if g == OB - 1:
                        nc.sync.dma_start(yt_r[:, g - 1], ot[:, 0])
                        nc.sync.dma_start(yt_r[:, g], ot[:, 1])
                    else:
                        nc.sync.dma_start(yt_r[:, g - 1 : g + 1], ot[:])

    nc.compile()
    return nc


def _run(inputs: dict, trace: bool = False):
    from concourse.bass_utils import run_bass_kernel_spmd

    xs = np.asarray(inputs["xs"])
    weight = np.asarray(inputs["weight"])
    bias = np.asarray(inputs["bias"], dtype=np.float32)

    lead_shape = xs.shape[:-1]
    xf = np.ascontiguousarray(xs.reshape(-1, D), dtype=np.float32)
    n_tok = xf.shape[0]
    assert n_tok % N_CORES == 0
    tpc = n_tok // N_CORES

    # host compose: dense butterfly matrix, scaled by DECAY
    w_dense = DECAY * _compose_dense(weight)[:, :OUT_DIM]
    # Power-of-2 rescale for fp8: the composed butterfly weights here are
    # ~2e-8 (the reference's normalization shrinks them ~1024x per layer),
    # far below e4m3's subnormal floor. Bring amax to ~2^7 on device and
    # undo it exactly (fp32 exponent shift) on the host after gathering.
    amax = float(np.abs(w_dense).max())
    exp = int(np.clip(np.floor(np.log2(128.0 / amax)), -120, 120)) if amax > 0 else 0
    scale = float(2.0 ** exp)
    w_dev = (w_dense * scale).astype(np.float32).astype(ml_dtypes.float8_e4m3)
    bias_dev = np.ascontiguousarray(bias * scale, dtype=np.float32)

    with_bias = bool(np.any(bias != 0.0))
    key = (tpc, exp, with_bias)
    if key not in _BASS_CACHE:
        _BASS_CACHE[key] = _build_bass(tpc, exp, with_bias=with_bias)
    nc = _BASS_CACHE[key]

    in_maps = []
    for c in range(N_CORES):
        xc = xf[c * tpc : (c + 1) * tpc]                    # [tpc, D] fp32
        in_maps.append(
            {
                "xb": np.ascontiguousarray(
                    ((1.0 - DECAY) * scale * xc).T
                ).astype(ml_dtypes.bfloat16),
                "w": w_dev,
                "bias": bias_dev,
            }
        )

    # The axon-tunneled terminal intermittently reports
    # NRT_EXEC_UNIT_UNRECOVERABLE; the immediately-following run always
    # succeeded. Retry with a backend reset.
    last_exc = None
    for attempt in range(3):
        try:
            res = run_bass_kernel_spmd(
                nc, in_maps, core_ids=list(range(N_CORES)), trace=trace
            )
            break
        except Exception as e:  # noqa: BLE001 - device fault -> reset + retry
            last_exc = e
            try:
                import jax
                import jax.extend

                jax.clear_caches()
                jax.extend.backend.clear_backends()
            except Exception:
                pass
    else:
        raise last_exc
    global LAST_EXEC_TIME_NS
    LAST_EXEC_TIME_NS = res.exec_time_ns

    y = np.concatenate(
        [np.asarray(r["yt"]).T.astype(np.float32) for r in res.results], axis=0
    )
    if scale != 1.0:
        y = y * np.float32(1.0 / scale)   # exact: power-of-2 exponent shift
    return y.reshape(*lead_shape, OUT_DIM), res


def kernel(**inputs) -> np.ndarray:
    out, _ = _run(inputs, trace=False)
    return out
